# revision 10
# baseline (speedup 1.0000x reference)
"""Trainium2 Bass kernel for nn_CausalTrajectoryPrediction.

Per-node stacked MLP over B=16384 rows, N=64 nodes:
  x1[b,i,:] = x[b,:] with entry i zeroed       (mask folded into weights host-side)
  z_i  = relu(W1a'_i @ x) , relu(W2a'_i @ x)   (two branches, packed M=128)
  r_i  = relu(blockdiag(W1b_i, W2b_i) @ z_i)   (K=128, M=64)
  h_i  = relu(W3ab_i @ r_i + w3x_i * x[:,i] + b3a_i)
  out  = relu(w3b_i . h_i + b3b_i)             (final bias+relu on host)

Layout: activations transposed [feature, B]; batch sharded across 8 cores
(BL=2048 each); nodes processed in pairs so every ACT/DVE op uses 128
partitions; matmul groups are subarray-tiled via tile_position for PE
concurrency.  Inputs arrive as 7 prepacked DRAM tensors (host does all
transposes/masking); built on Bacc so multi-semaphore waits are split
into EventSemaphores (walrus allows one wait per Matmult).
"""

import numpy as np
from contextlib import ExitStack

N, H, M, B = 64, 64, 32, 16384
NCORES = 8
BL = B // NCORES            # 2048 batch columns per core
CH = 512                    # chunk width (one PSUM bank of fp32)
NPAIR = N // 2              # 32 node pairs

_cache = {}


def _build_bass(bl, npair):
    import concourse.bass as bass
    import concourse.bacc as bacc
    import concourse.mybir as mybir
    import concourse.tile as tile

    F32 = mybir.dt.float32
    BF16 = mybir.dt.bfloat16
    Relu = mybir.ActivationFunctionType.Relu
    Copy = mybir.ActivationFunctionType.Copy
    nch = bl // CH

    nc = bacc.Bacc()
    xt_d = nc.dram_tensor("xt", [128, bl], BF16, kind="ExternalInput")
    w1_d = nc.dram_tensor("w1", [128, npair * 128], BF16, kind="ExternalInput")
    w2_d = nc.dram_tensor("w2", [128, npair * 128], BF16, kind="ExternalInput")
    w3_d = nc.dram_tensor("w3", [128, npair * 128], BF16, kind="ExternalInput")
    w3x_d = nc.dram_tensor("w3x", [N, npair * 128], BF16, kind="ExternalInput")
    w4_d = nc.dram_tensor("w4", [128, npair * 2], BF16, kind="ExternalInput")
    b3a_d = nc.dram_tensor("b3a", [128, npair], F32, kind="ExternalInput")
    out_d = nc.dram_tensor("opre", [bl, N], F32, kind="ExternalOutput")

    mm = nc.tensor.matmul  # bf16 operands: 1 PE cycle/row (fp32 was 4)

    with tile.TileContext(nc) as tc, ExitStack() as ctx:
        wpool = ctx.enter_context(tc.tile_pool(name="weights", bufs=1))
        apool = ctx.enter_context(tc.tile_pool(name="acts", bufs=2))
        ps_z = ctx.enter_context(tc.tile_pool(name="ps_z", bufs=2, space="PSUM"))
        ps_r = ctx.enter_context(tc.tile_pool(name="ps_r", bufs=2, space="PSUM"))
        ps_h = ctx.enter_context(tc.tile_pool(name="ps_h", bufs=1, space="PSUM"))
        ps_o = ctx.enter_context(tc.tile_pool(name="ps_o", bufs=1, space="PSUM"))

        # Parallel HWDGE loads; xt/w1 first so L1 compute starts ASAP.
        # (Bacc's generate_event_semaphores splits multi-waits, so matmuls
        # may depend on several DMA queues safely.)
        xt_sb = wpool.tile([128, bl], BF16, tag="xt")
        nc.sync.dma_start(xt_sb[:, 0:CH], xt_d[:, 0:CH])
        w1_sb = wpool.tile([128, npair * 128], BF16, tag="w1")
        nc.sync.dma_start(w1_sb[:, 0:512], w1_d[:, 0:512])
        nc.sync.dma_start(xt_sb[:, CH:bl], xt_d[:, CH:bl])
        nc.sync.dma_start(w1_sb[:, 512 : npair * 128], w1_d[:, 512 : npair * 128])
        w2_sb = wpool.tile([128, npair * 128], BF16, tag="w2")
        nc.sync.dma_start(w2_sb[:, 0:512], w2_d[:, 0:512])
        nc.sync.dma_start(w2_sb[:, 512 : npair * 128], w2_d[:, 512 : npair * 128])
        w3_sb = wpool.tile([128, npair * 128], BF16, tag="w3")
        nc.sync.dma_start(w3_sb[:, 0:512], w3_d[:, 0:512])
        nc.sync.dma_start(w3_sb[:, 512 : npair * 128], w3_d[:, 512 : npair * 128])
        w3x_sb = wpool.tile([N, npair * 128], BF16, tag="w3x")
        nc.sync.dma_start(w3x_sb[:], w3x_d[:])
        w4_sb = wpool.tile([128, npair * 2], BF16, tag="w4")
        nc.sync.dma_start(w4_sb[:], w4_d[:])
        b3a_sb = wpool.tile([128, npair], F32, tag="b3a")
        nc.sync.dma_start(b3a_sb[:], b3a_d[:])

        # PE warmup: ~12 dummy N=512 matmuls (~5us) run during the initial
        # DMA wait so the HAM clock gate reaches K=8/8 (2.4 GHz) before the
        # real work starts; a cold PE runs every stream at 1.2 GHz.
        wscr = wpool.tile([128, CH], BF16, tag="warm")
        nc.vector.memset(wscr[:], 0.0)
        scr_ps = ps_h.tile([128, CH], F32, tag="h", name="warm_ps")
        for _ in range(12):
            mm(scr_ps[:], wscr[:, 0:128], wscr[:], start=True, stop=True)

        iters = [(c, t) for c in range(nch) for t in range(npair)]
        obanks = {}

        def emit_L1(c, t):
            # L1: both branches for each node of the pair (K=64, M=128);
            # the two nodes run row-concurrent on the PE (xt duplicated at
            # partitions 64-127)
            xt_c = xt_sb[:, c * CH : (c + 1) * CH]
            w1t = w1_sb[:, t * 128 : (t + 1) * 128]
            z_ps = ps_z.tile([128, 2 * CH], F32, tag="z", name=f"z_{c}_{t}")
            mm(z_ps[:, 0:CH], w1t[0:64, :], xt_c[0:64, :], start=True, stop=True,
                             tile_position=(0, 0))
            mm(z_ps[:, CH : 2 * CH], w1t[64:128, :], xt_c[64:128, :], start=True, stop=True,
                             tile_position=(64, 0))
            z_sb = apool.tile([128, 2 * CH], BF16, tag="zsb", name=f"zsb_{c}_{t}")
            nc.vector.tensor_scalar_max(z_sb[:], z_ps[:], 0.0)
            return z_sb

        def emit_L4(c, t, g_sb):
            # L4 transposed: lhsT=g (M=128 batch cols), rhs=w4 (N=2) ->
            # out [b, node] with nodes on the PSUM free axis
            if c not in obanks:
                obanks[c] = (
                    ps_o.tile([128, 4 * N], F32, tag="o", name=f"o_{c}"),
                    apool.tile([128, 4 * N], F32, tag="osb", name=f"osb_{c}"))
            o_bank, o_sb = obanks[c]
            w4t = w4_sb[:, t * 2 : (t + 1) * 2]
            for bb in range(4):
                mm(
                    o_bank[:, bb * N + 2 * t : bb * N + 2 * t + 2],
                    g_sb[:, bb * 128 : (bb + 1) * 128],
                    w4t[:],
                    start=True, stop=True)
            if t == npair - 1:
                nc.scalar.activation(o_sb[:], o_bank[:], Copy)
                nc.sync.dma_start(
                    out_d[c * CH : (c + 1) * CH, :].rearrange(
                        "(bb p) n -> p bb n", p=128),
                    o_sb[:].rearrange("p (bb n) -> p bb n", n=N))

        z_cur = emit_L1(*iters[0])
        pend_L4 = None  # (c, t, g_sb) awaiting emission
        for k, (c, t) in enumerate(iters):
            cs = slice(c * CH, (c + 1) * CH)
            xt_c = xt_sb[:, cs]
            w2t = w2_sb[:, t * 128 : (t + 1) * 128]
            w3t = w3_sb[:, t * 128 : (t + 1) * 128]
            w3xt2 = w3x_sb[:, t * 128 : (t + 1) * 128]
            z_sb = z_cur

            # L2: block-diag (K=128, M=64) per node, packed into one PSUM
            r_ps = ps_r.tile([128, CH], F32, tag="r")
            mm(r_ps[0:64, :], w2t[:, 0:64], z_sb[:, 0:CH], start=True, stop=True,
                             tile_position=(0, 0))
            mm(r_ps[64:128, :], w2t[:, 64:128], z_sb[:, CH : 2 * CH], start=True, stop=True,
                             tile_position=(0, 64))
            f_sb = apool.tile([128, CH], BF16, tag="f")
            nc.scalar.activation(f_sb[:], r_ps[:], Relu)

            if pend_L4 is not None:
                emit_L4(*pend_L4)

            # L3: per-node mains (col-split quadrants), then the x2 term
            h_ps = ps_h.tile([128, CH], F32, tag="h")
            mm(h_ps[0:64, :], w3t[0:64, 0:64], f_sb[0:64, :], start=True, stop=False, skip_group_check=True,
               tile_position=(0, 0))
            mm(h_ps[64:128, :], w3t[64:128, 64:128], f_sb[64:128, :], start=True, stop=False, skip_group_check=True,
               tile_position=(64, 64))
            mm(h_ps[:], w3xt2[:], xt_c[0:64, :], start=False, stop=True, skip_group_check=True)
            g_sb = apool.tile([128, CH], BF16, tag="g")
            nc.scalar.activation(g_sb[:], h_ps[:], Relu, bias=b3a_sb[:, t : t + 1])

            if k + 1 < len(iters):
                z_cur = emit_L1(*iters[k + 1])
            pend_L4 = (c, t, g_sb)
        emit_L4(*pend_L4)

    nc.compile()
    return nc


def _prep_weights(W1a, W1b, W2a, W2b, W3a, b3a, W3b, npair=NPAIR):
    import ml_dtypes
    n = W1a.shape[0]
    mask = (1.0 - np.eye(n, dtype=np.float32))  # [i, n]
    W1am = W1a * mask[:, None, :]
    W2am = W2a * mask[:, None, :]
    w1 = np.zeros((npair, 128, 128), np.float32)
    w2 = np.zeros((npair, 128, 128), np.float32)
    w3 = np.zeros((npair, 128, 128), np.float32)
    w3x = np.zeros((npair, n, 128), np.float32)
    w4 = np.zeros((npair, 128, 2), np.float32)
    b3ap = np.zeros((128, npair), np.float32)
    for t in range(npair):
        i0, i1 = 2 * t, 2 * t + 1
        w1[t, 0:64, 0:64] = W1am[i0].T
        w1[t, 0:64, 64:128] = W2am[i0].T
        w1[t, 64:128, 0:64] = W1am[i1].T
        w1[t, 64:128, 64:128] = W2am[i1].T
        w2[t, 0:64, 0:32] = W1b[i0].T
        w2[t, 64:128, 32:64] = W2b[i0].T
        w2[t, 0:64, 64:96] = W1b[i1].T
        w2[t, 64:128, 96:128] = W2b[i1].T
        w3[t, 0:64, 0:64] = W3a[i0][:, 0:64].T
        w3[t, 64:128, 64:128] = W3a[i1][:, 0:64].T
        w3x[t, i0, 0:64] = W3a[i0][:, 64 + i0]
        w3x[t, i1, 64:128] = W3a[i1][:, 64 + i1]
        w4[t, 0:64, 0] = W3b[i0, 0]
        w4[t, 64:128, 1] = W3b[i1, 0]
        b3ap[0:64, t] = b3a[i0]
        b3ap[64:128, t] = b3a[i1]
    # pack pair-major arrays into the SBUF layout [P, npair*F], bf16
    pk = lambda a: np.ascontiguousarray(
        a.transpose(1, 0, 2).reshape(a.shape[1], -1)).astype(ml_dtypes.bfloat16)
    return pk(w1), pk(w2), pk(w3), pk(w3x), pk(w4), b3ap


def kernel(x, W1a, W1b, W2a, W2b, W3a, b3a, W3b, b3b):
    from concourse.bass_utils import run_bass_kernel_spmd

    x = np.asarray(x, np.float32)
    w1, w2, w3, w3x, w4, b3ap = _prep_weights(
        np.asarray(W1a, np.float32), np.asarray(W1b, np.float32),
        np.asarray(W2a, np.float32), np.asarray(W2b, np.float32),
        np.asarray(W3a, np.float32), np.asarray(b3a, np.float32),
        np.asarray(W3b, np.float32))
    b3b = np.asarray(b3b, np.float32)

    if "nc" not in _cache:
        _cache["nc"] = _build_bass(BL, NPAIR)
    nc = _cache["nc"]

    in_maps = []
    import ml_dtypes
    for core in range(NCORES):
        xs = x[core * BL : (core + 1) * BL]            # [BL, 64]
        xt = np.ascontiguousarray(
            np.concatenate([xs.T, xs.T], axis=0)).astype(ml_dtypes.bfloat16)
        in_maps.append({"xt": xt, "w1": w1, "w2": w2, "w3": w3,
                        "w3x": w3x, "w4": w4, "b3a": b3ap})

    res = run_bass_kernel_spmd(nc, in_maps, core_ids=list(range(NCORES)))
    out = np.empty((B, N), np.float32)
    for core in range(NCORES):
        opre = res.results[core]["opre"]               # [BL, 64]
        out[core * BL : (core + 1) * BL] = np.maximum(opre + b3b[:, 0][None, :], 0.0)
    return out



# revision 14
# speedup vs baseline: 1.5719x; 1.5719x over previous
"""Trainium2 Bass kernel for nn_CausalTrajectoryPrediction.

Per-node stacked MLP over B=16384 rows, N=64 nodes:
  x1[b,i,:] = x[b,:] with entry i zeroed       (mask folded into weights host-side)
  z_i  = relu(W1a'_i @ x) , relu(W2a'_i @ x)   (two branches, packed M=128)
  r_i  = relu(blockdiag(W1b_i, W2b_i) @ z_i)   (K=128, M=64)
  h_i  = relu(W3ab_i @ r_i + w3x_i * x[:,i] + b3a_i)
  out  = relu(w3b_i . h_i + b3b_i)             (final bias+relu on host)

Layout: activations transposed [feature, B]; batch sharded across 8 cores
(BL=2048 each); nodes processed in pairs so every ACT/DVE op uses 128
partitions; matmul groups are subarray-tiled via tile_position for PE
concurrency.  Inputs arrive as 7 prepacked DRAM tensors (host does all
transposes/masking); built on Bacc so multi-semaphore waits are split
into EventSemaphores (walrus allows one wait per Matmult).
"""

import numpy as np
from contextlib import ExitStack

N, H, M, B = 64, 64, 32, 16384
NCORES = 8
BL = B // NCORES            # 2048 batch columns per core
CH = 512                    # chunk width (one PSUM bank of fp32)
NPAIR = N // 2              # 32 node pairs

_cache = {}


def _build_bass(bl, npair):
    import concourse.bass as bass
    import concourse.bacc as bacc
    import concourse.mybir as mybir
    import concourse.tile as tile

    F32 = mybir.dt.float32
    BF16 = mybir.dt.bfloat16
    Relu = mybir.ActivationFunctionType.Relu
    Copy = mybir.ActivationFunctionType.Copy
    nch = bl // CH

    nc = bacc.Bacc()
    xt_d = nc.dram_tensor("xt", [128, bl], BF16, kind="ExternalInput")
    w1_d = nc.dram_tensor("w1", [128, npair * 128], BF16, kind="ExternalInput")
    w2_d = nc.dram_tensor("w2", [128, npair * 128], BF16, kind="ExternalInput")
    w3_d = nc.dram_tensor("w3", [128, npair * 128], BF16, kind="ExternalInput")
    w3x_d = nc.dram_tensor("w3x", [128, npair * 128], BF16, kind="ExternalInput")
    w4_d = nc.dram_tensor("w4", [128, npair * 2], BF16, kind="ExternalInput")
    b3a_d = nc.dram_tensor("b3a", [128, npair], F32, kind="ExternalInput")
    out_d = nc.dram_tensor("opre", [bl, N], F32, kind="ExternalOutput")

    mm = nc.tensor.matmul  # bf16 operands: 1 PE cycle/row (fp32 was 4)

    with tile.TileContext(nc) as tc, ExitStack() as ctx:
        wpool = ctx.enter_context(tc.tile_pool(name="weights", bufs=1))
        apool = ctx.enter_context(tc.tile_pool(name="acts", bufs=2))
        ps_z = ctx.enter_context(tc.tile_pool(name="ps_z", bufs=2, space="PSUM"))
        ps_r = ctx.enter_context(tc.tile_pool(name="ps_r", bufs=2, space="PSUM"))
        ps_h = ctx.enter_context(tc.tile_pool(name="ps_h", bufs=1, space="PSUM"))
        ps_o = ctx.enter_context(tc.tile_pool(name="ps_o", bufs=1, space="PSUM"))

        # Parallel HWDGE loads; xt/w1 first so L1 compute starts ASAP.
        # (Bacc's generate_event_semaphores splits multi-waits, so matmuls
        # may depend on several DMA queues safely.)
        xt_sb = wpool.tile([128, bl], BF16, tag="xt")
        nc.sync.dma_start(xt_sb[:, 0:CH], xt_d[:, 0:CH])
        w1_sb = wpool.tile([128, npair * 128], BF16, tag="w1")
        nc.sync.dma_start(w1_sb[:, 0:512], w1_d[:, 0:512])
        nc.sync.dma_start(xt_sb[:, CH:bl], xt_d[:, CH:bl])
        nc.sync.dma_start(w1_sb[:, 512 : npair * 128], w1_d[:, 512 : npair * 128])
        w2_sb = wpool.tile([128, npair * 128], BF16, tag="w2")
        nc.sync.dma_start(w2_sb[:, 0:512], w2_d[:, 0:512])
        nc.sync.dma_start(w2_sb[:, 512 : npair * 128], w2_d[:, 512 : npair * 128])
        w3_sb = wpool.tile([128, npair * 128], BF16, tag="w3")
        nc.sync.dma_start(w3_sb[:, 0:512], w3_d[:, 0:512])
        nc.sync.dma_start(w3_sb[:, 512 : npair * 128], w3_d[:, 512 : npair * 128])
        w3x_sb = wpool.tile([128, npair * 128], BF16, tag="w3x")
        nc.sync.dma_start(w3x_sb[:], w3x_d[:])
        w4_sb = wpool.tile([128, npair * 2], BF16, tag="w4")
        nc.sync.dma_start(w4_sb[:], w4_d[:])
        b3a_sb = wpool.tile([128, npair], F32, tag="b3a")
        nc.sync.dma_start(b3a_sb[:], b3a_d[:])

        iters = [(c, t) for c in range(nch) for t in range(npair)]
        obanks = {}

        def emit_L1(c, t):
            # L1: both branches for each node of the pair (K=64, M=128);
            # the two nodes run row-concurrent on the PE (xt duplicated at
            # partitions 64-127)
            xt_c = xt_sb[:, c * CH : (c + 1) * CH]
            w1t = w1_sb[:, t * 128 : (t + 1) * 128]
            z_ps = ps_z.tile([128, 2 * CH], F32, tag="z", name=f"z_{c}_{t}")
            mm(z_ps[:, 0:CH], w1t[0:64, :], xt_c[0:64, :], start=True, stop=True,
                             tile_position=(0, 0))
            mm(z_ps[:, CH : 2 * CH], w1t[64:128, :], xt_c[64:128, :], start=True, stop=True,
                             tile_position=(64, 0))
            z_sb = apool.tile([128, 2 * CH], BF16, tag="zsb", name=f"zsb_{c}_{t}")
            nc.vector.tensor_scalar_max(z_sb[:], z_ps[:], 0.0)
            return z_sb

        def emit_L4(c, t, g_sb):
            # L4 transposed: lhsT=g (M=128 batch cols), rhs=w4 (N=2) ->
            # out [b, node] with nodes on the PSUM free axis
            if c not in obanks:
                obanks[c] = (
                    ps_o.tile([128, 4 * N], F32, tag="o", name=f"o_{c}"),
                    apool.tile([128, 4 * N], F32, tag="osb", name=f"osb_{c}"))
            o_bank, o_sb = obanks[c]
            w4t = w4_sb[:, t * 2 : (t + 1) * 2]
            for bb in range(4):
                mm(
                    o_bank[:, bb * N + 2 * t : bb * N + 2 * t + 2],
                    g_sb[:, bb * 128 : (bb + 1) * 128],
                    w4t[:],
                    start=True, stop=True)
            if t == npair - 1:
                nc.scalar.activation(o_sb[:], o_bank[:], Copy)
                nc.sync.dma_start(
                    out_d[c * CH : (c + 1) * CH, :].rearrange(
                        "(bb p) n -> p bb n", p=128),
                    o_sb[:].rearrange("p (bb n) -> p bb n", n=N))

        z_cur = emit_L1(*iters[0])
        pend_L4 = None  # (c, t, g_sb) awaiting emission
        for k, (c, t) in enumerate(iters):
            cs = slice(c * CH, (c + 1) * CH)
            xt_c = xt_sb[:, cs]
            w2t = w2_sb[:, t * 128 : (t + 1) * 128]
            w3t = w3_sb[:, t * 128 : (t + 1) * 128]
            w3xt2 = w3x_sb[:, t * 128 : (t + 1) * 128]
            z_sb = z_cur

            # L2: block-diag (K=128, M=64) per node, packed into one PSUM
            r_ps = ps_r.tile([128, CH], F32, tag="r")
            mm(r_ps[0:64, :], w2t[:, 0:64], z_sb[:, 0:CH], start=True, stop=True,
                             tile_position=(0, 0))
            mm(r_ps[64:128, :], w2t[:, 64:128], z_sb[:, CH : 2 * CH], start=True, stop=True,
                             tile_position=(0, 64))
            f_sb = apool.tile([128, CH], BF16, tag="f")
            nc.scalar.activation(f_sb[:], r_ps[:], Relu)

            if pend_L4 is not None:
                emit_L4(*pend_L4)

            # L3: per-node mains (col-split quadrants), then the x2 term
            h_ps = ps_h.tile([128, CH], F32, tag="h")
            # x-terms start the accumulation (diagonal 64x64 quadrants, one
            # concurrent window), mains stop it (second window): 2 windows
            # for 4 mms instead of 3.
            mm(h_ps[0:64, :], w3xt2[0:64, 0:64], xt_c[0:64, :], start=True, stop=False, skip_group_check=True,
               tile_position=(0, 0))
            mm(h_ps[64:128, :], w3xt2[64:128, 64:128], xt_c[64:128, :], start=True, stop=False, skip_group_check=True,
               tile_position=(64, 64))
            mm(h_ps[0:64, :], w3t[0:64, 0:64], f_sb[0:64, :], start=False, stop=True, skip_group_check=True,
               tile_position=(0, 0))
            mm(h_ps[64:128, :], w3t[64:128, 64:128], f_sb[64:128, :], start=False, stop=True, skip_group_check=True,
               tile_position=(64, 64))
            g_sb = apool.tile([128, CH], BF16, tag="g")
            nc.scalar.activation(g_sb[:], h_ps[:], Relu, bias=b3a_sb[:, t : t + 1])

            if k + 1 < len(iters):
                z_cur = emit_L1(*iters[k + 1])
            pend_L4 = (c, t, g_sb)
        emit_L4(*pend_L4)

    nc.compile()
    return nc


def _prep_weights(W1a, W1b, W2a, W2b, W3a, b3a, W3b, npair=NPAIR):
    import ml_dtypes
    n = W1a.shape[0]
    mask = (1.0 - np.eye(n, dtype=np.float32))  # [i, n]
    W1am = W1a * mask[:, None, :]
    W2am = W2a * mask[:, None, :]
    w1 = np.zeros((npair, 128, 128), np.float32)
    w2 = np.zeros((npair, 128, 128), np.float32)
    w3 = np.zeros((npair, 128, 128), np.float32)
    w3x = np.zeros((npair, 128, 128), np.float32)
    w4 = np.zeros((npair, 128, 2), np.float32)
    b3ap = np.zeros((128, npair), np.float32)
    for t in range(npair):
        i0, i1 = 2 * t, 2 * t + 1
        w1[t, 0:64, 0:64] = W1am[i0].T
        w1[t, 0:64, 64:128] = W2am[i0].T
        w1[t, 64:128, 0:64] = W1am[i1].T
        w1[t, 64:128, 64:128] = W2am[i1].T
        w2[t, 0:64, 0:32] = W1b[i0].T
        w2[t, 64:128, 32:64] = W2b[i0].T
        w2[t, 0:64, 64:96] = W1b[i1].T
        w2[t, 64:128, 96:128] = W2b[i1].T
        w3[t, 0:64, 0:64] = W3a[i0][:, 0:64].T
        w3[t, 64:128, 64:128] = W3a[i1][:, 0:64].T
        w3x[t, i0, 0:64] = W3a[i0][:, 64 + i0]
        w3x[t, 64 + i0, 0:64] = W3a[i0][:, 64 + i0]
        w3x[t, i1, 64:128] = W3a[i1][:, 64 + i1]
        w3x[t, 64 + i1, 64:128] = W3a[i1][:, 64 + i1]
        w4[t, 0:64, 0] = W3b[i0, 0]
        w4[t, 64:128, 1] = W3b[i1, 0]
        b3ap[0:64, t] = b3a[i0]
        b3ap[64:128, t] = b3a[i1]
    # pack pair-major arrays into the SBUF layout [P, npair*F], bf16
    pk = lambda a: np.ascontiguousarray(
        a.transpose(1, 0, 2).reshape(a.shape[1], -1)).astype(ml_dtypes.bfloat16)
    return pk(w1), pk(w2), pk(w3), pk(w3x), pk(w4), b3ap


def kernel(x, W1a, W1b, W2a, W2b, W3a, b3a, W3b, b3b):
    from concourse.bass_utils import run_bass_kernel_spmd

    x = np.asarray(x, np.float32)
    w1, w2, w3, w3x, w4, b3ap = _prep_weights(
        np.asarray(W1a, np.float32), np.asarray(W1b, np.float32),
        np.asarray(W2a, np.float32), np.asarray(W2b, np.float32),
        np.asarray(W3a, np.float32), np.asarray(b3a, np.float32),
        np.asarray(W3b, np.float32))
    b3b = np.asarray(b3b, np.float32)

    if "nc" not in _cache:
        _cache["nc"] = _build_bass(BL, NPAIR)
    nc = _cache["nc"]

    in_maps = []
    import ml_dtypes
    for core in range(NCORES):
        xs = x[core * BL : (core + 1) * BL]            # [BL, 64]
        xt = np.ascontiguousarray(
            np.concatenate([xs.T, xs.T], axis=0)).astype(ml_dtypes.bfloat16)
        in_maps.append({"xt": xt, "w1": w1, "w2": w2, "w3": w3,
                        "w3x": w3x, "w4": w4, "b3a": b3ap})

    res = run_bass_kernel_spmd(nc, in_maps, core_ids=list(range(NCORES)))
    out = np.empty((B, N), np.float32)
    for core in range(NCORES):
        opre = res.results[core]["opre"]               # [BL, 64]
        out[core * BL : (core + 1) * BL] = np.maximum(opre + b3b[:, 0][None, :], 0.0)
    return out



# revision 16
# speedup vs baseline: 1.5814x; 1.0060x over previous
"""Trainium2 Bass kernel for nn_CausalTrajectoryPrediction.

Per-node stacked MLP over B=16384 rows, N=64 nodes:
  x1[b,i,:] = x[b,:] with entry i zeroed       (mask folded into weights host-side)
  z_i  = relu(W1a'_i @ x) , relu(W2a'_i @ x)   (two branches, packed M=128)
  r_i  = relu(blockdiag(W1b_i, W2b_i) @ z_i)   (K=128, M=64)
  h_i  = relu(W3ab_i @ r_i + w3x_i * x[:,i] + b3a_i)
  out  = relu(w3b_i . h_i + b3b_i)             (final bias+relu on host)

Layout: activations transposed [feature, B]; batch sharded across 8 cores
(BL=2048 each); nodes processed in pairs so every ACT/DVE op uses 128
partitions; matmul groups are subarray-tiled via tile_position for PE
concurrency.  Inputs arrive as 7 prepacked DRAM tensors (host does all
transposes/masking); built on Bacc so multi-semaphore waits are split
into EventSemaphores (walrus allows one wait per Matmult).
"""

import numpy as np
from contextlib import ExitStack

N, H, M, B = 64, 64, 32, 16384
NCORES = 8
BL = B // NCORES            # 2048 batch columns per core
CH = 512                    # chunk width (one PSUM bank of fp32)
NPAIR = N // 2              # 32 node pairs

_cache = {}


def _build_bass(bl, npair):
    import concourse.bass as bass
    import concourse.bacc as bacc
    import concourse.mybir as mybir
    import concourse.tile as tile

    F32 = mybir.dt.float32
    BF16 = mybir.dt.bfloat16
    Relu = mybir.ActivationFunctionType.Relu
    Copy = mybir.ActivationFunctionType.Copy
    nch = bl // CH

    nc = bacc.Bacc()
    xt_d = nc.dram_tensor("xt", [128, bl], BF16, kind="ExternalInput")
    w1_d = nc.dram_tensor("w1", [128, npair * 128], BF16, kind="ExternalInput")
    w2_d = nc.dram_tensor("w2", [128, npair * 128], BF16, kind="ExternalInput")
    w3_d = nc.dram_tensor("w3", [128, npair * 128], BF16, kind="ExternalInput")
    w3x_d = nc.dram_tensor("w3x", [128, npair * 128], BF16, kind="ExternalInput")
    w4_d = nc.dram_tensor("w4", [128, npair * 2], BF16, kind="ExternalInput")
    b3a_d = nc.dram_tensor("b3a", [128, npair], F32, kind="ExternalInput")
    out_d = nc.dram_tensor("opre", [bl, N], F32, kind="ExternalOutput")

    mm = nc.tensor.matmul  # bf16 operands: 1 PE cycle/row (fp32 was 4)

    with tile.TileContext(nc) as tc, ExitStack() as ctx:
        wpool = ctx.enter_context(tc.tile_pool(name="weights", bufs=1))
        apool = ctx.enter_context(tc.tile_pool(name="acts", bufs=2))
        ps_z = ctx.enter_context(tc.tile_pool(name="ps_z", bufs=2, space="PSUM"))
        ps_r = ctx.enter_context(tc.tile_pool(name="ps_r", bufs=2, space="PSUM"))
        ps_h = ctx.enter_context(tc.tile_pool(name="ps_h", bufs=1, space="PSUM"))
        ps_o = ctx.enter_context(tc.tile_pool(name="ps_o", bufs=1, space="PSUM"))

        # Parallel HWDGE loads; xt/w1 first so L1 compute starts ASAP.
        # (Bacc's generate_event_semaphores splits multi-waits, so matmuls
        # may depend on several DMA queues safely.)
        xt_sb = wpool.tile([128, bl], BF16, tag="xt")
        nc.sync.dma_start(xt_sb[:, 0:CH], xt_d[:, 0:CH])
        w1_sb = wpool.tile([128, npair * 128], BF16, tag="w1")
        nc.sync.dma_start(w1_sb[:, 0:512], w1_d[:, 0:512])
        nc.sync.dma_start(xt_sb[:, CH:bl], xt_d[:, CH:bl])
        nc.sync.dma_start(w1_sb[:, 512 : npair * 128], w1_d[:, 512 : npair * 128])
        w2_sb = wpool.tile([128, npair * 128], BF16, tag="w2")
        nc.sync.dma_start(w2_sb[:, 0:512], w2_d[:, 0:512])
        nc.sync.dma_start(w2_sb[:, 512 : npair * 128], w2_d[:, 512 : npair * 128])
        w3_sb = wpool.tile([128, npair * 128], BF16, tag="w3")
        nc.sync.dma_start(w3_sb[:, 0:512], w3_d[:, 0:512])
        nc.sync.dma_start(w3_sb[:, 512 : npair * 128], w3_d[:, 512 : npair * 128])
        w3x_sb = wpool.tile([128, npair * 128], BF16, tag="w3x")
        nc.sync.dma_start(w3x_sb[:], w3x_d[:])
        w4_sb = wpool.tile([128, npair * 2], BF16, tag="w4")
        nc.sync.dma_start(w4_sb[:], w4_d[:])
        b3a_sb = wpool.tile([128, npair], F32, tag="b3a")
        nc.sync.dma_start(b3a_sb[:], b3a_d[:])

        iters = [(c, t) for c in range(nch) for t in range(npair)]
        obanks = {}

        def emit_L1(c, t):
            # L1: both branches for each node of the pair (K=64, M=128);
            # the two nodes run row-concurrent on the PE (xt duplicated at
            # partitions 64-127)
            xt_c = xt_sb[:, c * CH : (c + 1) * CH]
            w1t = w1_sb[:, t * 128 : (t + 1) * 128]
            z_ps = ps_z.tile([128, 2 * CH], F32, tag="z", name=f"z_{c}_{t}")
            mm(z_ps[:, 0:CH], w1t[0:64, :], xt_c[0:64, :], start=True, stop=True,
                             tile_position=(0, 0))
            mm(z_ps[:, CH : 2 * CH], w1t[64:128, :], xt_c[64:128, :], start=True, stop=True,
                             tile_position=(64, 0))
            z_sb = apool.tile([128, 2 * CH], BF16, tag="zsb", name=f"zsb_{c}_{t}")
            nc.vector.tensor_scalar_max(z_sb[:], z_ps[:], 0.0)
            return z_sb

        def emit_L4(c, t, g_sb):
            # L4 transposed: lhsT=g (M=128 batch cols), rhs=w4 (N=2) ->
            # out [b, node] with nodes on the PSUM free axis
            if c not in obanks:
                obanks[c] = (
                    ps_o.tile([128, 4 * N], F32, tag="o", name=f"o_{c}"),
                    apool.tile([128, 4 * N], F32, tag="osb", name=f"osb_{c}"))
            o_bank, o_sb = obanks[c]
            w4t = w4_sb[:, t * 2 : (t + 1) * 2]
            for bb in range(4):
                mm(
                    o_bank[:, bb * N + 2 * t : bb * N + 2 * t + 2],
                    g_sb[:, bb * 128 : (bb + 1) * 128],
                    w4t[:],
                    start=True, stop=True)
            if t == npair - 1:
                nc.scalar.activation(o_sb[:], o_bank[:], Copy)
                nc.sync.dma_start(
                    out_d[c * CH : (c + 1) * CH, :].rearrange(
                        "(bb p) n -> p bb n", p=128),
                    o_sb[:].rearrange("p (bb n) -> p bb n", n=N))

        z_cur = emit_L1(*iters[0])
        pend_L4 = None  # (c, t, g_sb) awaiting emission
        for k, (c, t) in enumerate(iters):
            cs = slice(c * CH, (c + 1) * CH)
            xt_c = xt_sb[:, cs]
            w2t = w2_sb[:, t * 128 : (t + 1) * 128]
            w3t = w3_sb[:, t * 128 : (t + 1) * 128]
            w3xt2 = w3x_sb[:, t * 128 : (t + 1) * 128]
            z_sb = z_cur

            # L2: block-diag (K=128, M=64) per node, packed into one PSUM
            r_ps = ps_r.tile([128, CH], F32, tag="r")
            mm(r_ps[0:64, :], w2t[:, 0:64], z_sb[:, 0:CH], start=True, stop=True,
                             tile_position=(0, 0))
            mm(r_ps[64:128, :], w2t[:, 64:128], z_sb[:, CH : 2 * CH], start=True, stop=True,
                             tile_position=(0, 64))
            f_sb = apool.tile([128, CH], BF16, tag="f")
            nc.scalar.activation(f_sb[:], r_ps[:], Relu)

            if pend_L4 is not None:
                emit_L4(*pend_L4)

            # L3: per-node mains (col-split quadrants), then the x2 term
            h_ps = ps_h.tile([128, CH], F32, tag="h")
            # x-terms start the accumulation (diagonal 64x64 quadrants, one
            # concurrent window), mains stop it (second window): 2 windows
            # for 4 mms instead of 3.
            mm(h_ps[0:64, :], w3xt2[0:64, 0:64], xt_c[0:64, :], start=True, stop=False, skip_group_check=True,
               tile_position=(0, 0))
            mm(h_ps[64:128, :], w3xt2[64:128, 64:128], xt_c[64:128, :], start=True, stop=False, skip_group_check=True,
               tile_position=(64, 64))
            mm(h_ps[0:64, :], w3t[0:64, 0:64], f_sb[0:64, :], start=False, stop=True, skip_group_check=True,
               tile_position=(0, 0))
            mm(h_ps[64:128, :], w3t[64:128, 64:128], f_sb[64:128, :], start=False, stop=True, skip_group_check=True,
               tile_position=(64, 64))
            g_sb = apool.tile([128, CH], BF16, tag="g")
            nc.scalar.activation(g_sb[:], h_ps[:], Relu, bias=b3a_sb[:, t : t + 1])

            if k + 1 < len(iters):
                z_cur = emit_L1(*iters[k + 1])
            pend_L4 = (c, t, g_sb)
        emit_L4(*pend_L4)

    nc.compile()
    return nc


def _prep_weights(W1a, W1b, W2a, W2b, W3a, b3a, W3b, npair=NPAIR):
    import ml_dtypes
    n = W1a.shape[0]
    mask = (1.0 - np.eye(n, dtype=np.float32))  # [i, n]
    W1am = W1a * mask[:, None, :]
    W2am = W2a * mask[:, None, :]
    w1 = np.zeros((npair, 128, 128), np.float32)
    w2 = np.zeros((npair, 128, 128), np.float32)
    w3 = np.zeros((npair, 128, 128), np.float32)
    w3x = np.zeros((npair, 128, 128), np.float32)
    w4 = np.zeros((npair, 128, 2), np.float32)
    b3ap = np.zeros((128, npair), np.float32)
    for t in range(npair):
        i0, i1 = 2 * t, 2 * t + 1
        w1[t, 0:64, 0:64] = W1am[i0].T
        w1[t, 0:64, 64:128] = W2am[i0].T
        w1[t, 64:128, 0:64] = W1am[i1].T
        w1[t, 64:128, 64:128] = W2am[i1].T
        w2[t, 0:64, 0:32] = W1b[i0].T
        w2[t, 64:128, 32:64] = W2b[i0].T
        w2[t, 0:64, 64:96] = W1b[i1].T
        w2[t, 64:128, 96:128] = W2b[i1].T
        w3[t, 0:64, 0:64] = W3a[i0][:, 0:64].T
        w3[t, 64:128, 64:128] = W3a[i1][:, 0:64].T
        w3x[t, i0, 0:64] = W3a[i0][:, 64 + i0]
        w3x[t, 64 + i0, 0:64] = W3a[i0][:, 64 + i0]
        w3x[t, i1, 64:128] = W3a[i1][:, 64 + i1]
        w3x[t, 64 + i1, 64:128] = W3a[i1][:, 64 + i1]
        w4[t, 0:64, 0] = W3b[i0, 0]
        w4[t, 64:128, 1] = W3b[i1, 0]
        b3ap[0:64, t] = b3a[i0]
        b3ap[64:128, t] = b3a[i1]
    # pack pair-major arrays into the SBUF layout [P, npair*F], bf16
    pk = lambda a: np.ascontiguousarray(
        a.transpose(1, 0, 2).reshape(a.shape[1], -1)).astype(ml_dtypes.bfloat16)
    return pk(w1), pk(w2), pk(w3), pk(w3x), pk(w4), b3ap


def kernel(x, W1a, W1b, W2a, W2b, W3a, b3a, W3b, b3b):
    from concourse.bass_utils import run_bass_kernel_spmd

    x = np.asarray(x, np.float32)
    w1, w2, w3, w3x, w4, b3ap = _prep_weights(
        np.asarray(W1a, np.float32), np.asarray(W1b, np.float32),
        np.asarray(W2a, np.float32), np.asarray(W2b, np.float32),
        np.asarray(W3a, np.float32), np.asarray(b3a, np.float32),
        np.asarray(W3b, np.float32))
    b3b = np.asarray(b3b, np.float32)

    if "nc" not in _cache:
        _cache["nc"] = _build_bass(BL, NPAIR)
    nc = _cache["nc"]

    in_maps = []
    import ml_dtypes
    for core in range(NCORES):
        xs = x[core * BL : (core + 1) * BL]            # [BL, 64]
        xt = np.ascontiguousarray(
            np.concatenate([xs.T, xs.T], axis=0)).astype(ml_dtypes.bfloat16)
        in_maps.append({"xt": xt, "w1": w1, "w2": w2, "w3": w3,
                        "w3x": w3x, "w4": w4, "b3a": b3ap})

    res = run_bass_kernel_spmd(nc, in_maps, core_ids=list(range(NCORES)))
    out = np.empty((B, N), np.float32)
    for core in range(NCORES):
        opre = res.results[core]["opre"]               # [BL, 64]
        out[core * BL : (core + 1) * BL] = np.maximum(opre + b3b[:, 0][None, :], 0.0)
    return out

